# revision 9
# baseline (speedup 1.0000x reference)
"""Trainium2 Bass kernel for nn_DoG_Seasonal: depthwise Difference-of-Gaussians
1-D convolution along L with reflect padding.

Architecture (v2, split fine/coarse):
  y = x * (k1 - k2).  The combined 769-tap kernel is decomposed exactly-LS into
    fine:   radius-63 kernel F on 64-shifted 128-chunks -> 2 matmuls/tile
    coarse: wide sigma=96 part approximated on D=32 cells with a per-cell
            (mean, slope) basis; pooling = 1 matmul/chunk on PE; apply =
            1-2 matmuls/tile with contraction over 56 interleaved pooled rows.
  White-noise approximation error of the decomposition: 8.5e-4 (vs 2e-2 gate).

  Per batch matmuls: 64 fine + 34 pool + 38 apply = 136 (vs 160 baseline),
  all bf16 in v1.

Sharding: data-parallel over batch - 32 batches / 8 cores = 4 per core.
"""

import os as _os

import numpy as np
import ml_dtypes

import concourse.bacc as bacc
import concourse.mybir as mybir
import concourse.tile as tile
from concourse.bass_utils import run_bass_kernel_spmd

# ---- problem constants (hardcoded per harness contract) ----
B, L, C = 32, 4096, 321
N_CORES = 8
BPC = B // N_CORES            # batches per core
P = 128                       # partition / tile size
NTILES = L // P               # 32 output tiles per batch
NCHUNKS = NTILES + 1          # 33 shifted chunks (chunk k = [128k-64, 128k+64))
D = 32                        # coarse cell width
NCELLS = L // D               # 128 cells per batch
SIGMA1, SIGMA2, TRUNCATE = 4.2, 96.0, 4.0

PSG = 2                       # output tiles per PSUM group
OGRP = int(_os.environ.get("DOG_OGRP", "4"))      # output tiles per out-DMA
XBUFS = int(_os.environ.get("DOG_XBUFS", "14"))
OBUFS = int(_os.environ.get("DOG_OBUFS", "4"))
ACT_EVERY = int(_os.environ.get("DOG_ACT_EVERY", "3"))  # k-th evac on ScalarE

BF16 = ml_dtypes.bfloat16


# ---------------- host-side weight construction ----------------

def _gauss(sigma):
    r = int(TRUNCATE * sigma + 0.5)
    t = np.arange(-r, r + 1, dtype=np.float64)
    k = np.exp(-0.5 * (t / sigma) ** 2)
    return k / k.sum()


def _combined_kernel():
    k1, k2 = _gauss(SIGMA1), _gauss(SIGMA2)
    r1, r2 = (len(k1) - 1) // 2, (len(k2) - 1) // 2
    kc = -k2
    kc[r2 - r1: r2 + r1 + 1] += k1
    return kc, r2           # radius 384


def _exact_tile_operator(m, kc, r2):
    """W [L, P]: col t = exact reflected kernel row for output 128m+t."""
    o = np.arange(P)
    t = np.arange(len(kc))
    u = P * m + o[None, :] + t[:, None] - r2
    v = np.abs(u)
    v = np.where(v > L - 1, 2 * (L - 1) - v, v)
    W = np.zeros((L, P))
    np.add.at(W, (v.ravel(), np.broadcast_to(o[None, :], v.shape).ravel()),
              np.broadcast_to(kc[:, None], v.shape).ravel())
    return W


def _pool_basis():
    s = np.arange(D, dtype=np.float64)
    phi0 = np.full(D, 1.0 / D)
    u = (s - (D - 1) / 2.0) / ((D - 1) / 2.0)
    phi1 = u / D
    return np.stack([phi0, phi1])                     # [2, D]


def _build_tile(m, kc, r2, phi):
    """LS decomposition for tile m. Returns (F [256,P], j0, j1, Cw [2n,P])."""
    W = _exact_tile_operator(m, kc, r2)
    win_lo, win_hi = max(0, P * m - 64), min(L, P * m + 192)
    j0, j1 = max(0, 4 * m - 12), min(NCELLS, 4 * m + 16)
    inwin = np.zeros(L, dtype=bool)
    inwin[win_lo:win_hi] = True

    Cw = np.zeros(((j1 - j0) * 2, P))
    approx = np.zeros_like(W)
    Bmat = phi.T                                      # [D, 2]
    for ci, j in enumerate(range(j0, j1)):
        rows = np.arange(D * j, D * j + D)
        outside = ~inwin[rows]
        if outside.sum() == 0:
            continue
        Bout = Bmat[outside]
        G = Bout.T @ Bout
        rhs = Bout.T @ W[rows[outside], :]
        Cj = np.linalg.solve(G + 1e-18 * np.eye(2), rhs)
        Cw[2 * ci: 2 * ci + 2, :] = Cj
        approx[rows, :] += Bmat @ Cj
    F = np.zeros((256, P))
    lo = P * m - 64
    F[win_lo - lo: win_hi - lo, :] = (W - approx)[win_lo:win_hi, :]
    return F, j0, j1, Cw


class _Blocks:
    """Dedup store of weight blocks packed into one [128, ncols] DRAM tensor."""

    def __init__(self):
        self.keys = {}
        self.blocks = []      # list of np [128, ncols] float32
        self.offsets = []
        self.ncols = 0

    def add(self, arr):
        """arr [rows<=128, cols]; zero-pads partitions to 128. Returns id."""
        a = np.zeros((P, arr.shape[1]), dtype=np.float32)
        a[: arr.shape[0]] = arr
        a_bf = a.astype(BF16)
        key = a_bf.tobytes()
        if key not in self.keys:
            self.keys[key] = len(self.blocks)
            self.blocks.append(a_bf)
            self.offsets.append(self.ncols)
            self.ncols += a.shape[1]
        return self.keys[key]

    def flat(self):
        return np.concatenate(self.blocks, axis=1)    # [128, ncols] bf16


def _build_weights_and_schedule():
    """Returns (blocks, fine_sched, apply_sched, pool_sched).

    All matmuls use full-width stationary blocks with zeros baked in, so
    every partition base is 0 (or 64 for the two half-chunks) and every
    psum write covers a legal aligned range.

    fine_sched[m]  = [(blk, (r0, r1), chunk_k), ...]   # lhsT/rhs partition slice
    apply_sched[m] = [(blk, h, start), ...]            # contraction = full 128
    pool_sched     = [(k, (r0, r1), blk, h, start, stop), ...]  # out = full half
    """
    kc, r2 = _combined_kernel()
    phi = _pool_basis()
    blocks = _Blocks()

    fine_sched, apply_sched = [], []
    for m in range(NTILES):
        F, j0, j1, Cw = _build_tile(m, kc, r2, phi)
        fs = []
        rl = (64, 128) if m == 0 else (0, 128)
        fs.append((blocks.add(F[0:128]), rl, m))
        rr = (0, 64) if m == NTILES - 1 else (0, 128)
        fs.append((blocks.add(F[128:256]), rr, m + 1))
        fine_sched.append(fs)

        # apply: per half, full [128, P] lhsT with window rows at 2*(j-64h)+b
        pieces = []
        for h in range(2):
            ja, jb = max(j0, 64 * h), min(j1, 64 * h + 64)
            if jb <= ja:
                continue
            A = np.zeros((P, P))
            for j in range(ja, jb):
                for b in range(2):
                    A[2 * (j - 64 * h) + b, :] = Cw[2 * (j - j0) + b, :]
            pieces.append((blocks.add(A), h))
        apply_sched.append(pieces)

    # pool: chunk k covers cells 4k-2..4k+1 (clipped); lhsT [128 or 64, 128]
    # maps chunk partition p -> interleaved pooled row 2*(cell-64h)+b of half h.
    # Each matmul writes the full half (zeros elsewhere) and accumulates.
    pool_sched = []
    writers = {0: [], 1: []}
    for k in range(NCHUNKS):
        pos_lo = max(0, P * k - 64)
        pos_hi = min(L, P * k + 64)
        cells = range(pos_lo // D, (pos_hi - 1) // D + 1)
        by_half = {}
        for j in cells:
            by_half.setdefault(j // 64, []).append(j)
        for h, js in sorted(by_half.items()):
            A = np.zeros((P, P))
            for j in js:
                for p in range(P * k - 64 + 0, P * k + 64):
                    if p < D * j or p >= D * j + D or p < 0 or p >= L:
                        continue
                    part = p - (P * k - 64)
                    for b in range(2):
                        A[part, 2 * (j - 64 * h) + b] = phi[b][p % D]
            r = (64, 128) if k == 0 else (0, 64) if k == NCHUNKS - 1 else (0, 128)
            writers[h].append((k, r, blocks.add(A)))
    for h in range(2):
        n = len(writers[h])
        for i, (k, r, bid) in enumerate(writers[h]):
            pool_sched.append((k, r, bid, h, i == 0, i == n - 1))
    pool_sched.sort(key=lambda e: (e[0], e[3]))
    return blocks, fine_sched, apply_sched, pool_sched


# ---------------- device program ----------------

def _dedupe_ldweights(nc):
    """Drop InstLdweights that reload the identical weights AP (PE order)."""
    removed = 0
    for blk in nc.main_func.blocks:
        last_key = None
        new = []
        changed = False
        for inst in blk.instructions:
            nm = type(inst).__name__
            if nm == "InstLdweights":
                key = str(inst.ins[0])
                si = inst.sync_info
                clean = si is None or (len(si.on_wait) == 0 and len(si.on_update) == 0)
                if key == last_key and clean:
                    removed += 1
                    changed = True
                    continue
                last_key = key
            elif nm == "InstMatmult":
                pass
            elif getattr(inst, "engine", None) == mybir.EngineType.PE:
                last_key = None
            new.append(inst)
        if changed:
            blk.instructions = new
    return removed


def _build_program(blocks, fine_sched, apply_sched, pool_sched, repeat=1):
    ncols = blocks.ncols
    offs = blocks.offsets
    _PREV_MM = [None]
    nc = bacc.Bacc(None, target_bir_lowering=False)
    x_d = nc.declare_dram_parameter("x", [BPC * L, C], mybir.dt.bfloat16, isOutput=False)
    w_d = nc.declare_dram_parameter("w", [P, ncols], mybir.dt.bfloat16, isOutput=False)
    out_d = nc.declare_dram_parameter("out", [BPC * L, C], mybir.dt.bfloat16, isOutput=True)

    with tile.TileContext(nc) as tc:
        with (
            tc.tile_pool(name="wpool", bufs=1) as wpool,
            tc.tile_pool(name="xpool", bufs=XBUFS) as xpool,
            tc.tile_pool(name="plpool", bufs=4) as plpool,
            tc.tile_pool(name="opool", bufs=OBUFS) as opool,
            tc.tile_pool(name="psout", bufs=3, space="PSUM") as psout,
            tc.tile_pool(name="pspool", bufs=1, space="PSUM") as pspool,
        ):
            # weights: split so early blocks (pool + first tiles) land first
            wsplit = 0
            for e in pool_sched[:4] + fine_sched[0] + fine_sched[1]:
                wsplit = max(wsplit, offs[e[2] if len(e) == 6 else e[0]] + P)
            for e in apply_sched[0] + apply_sched[1]:
                wsplit = max(wsplit, offs[e[0]] + P)
            w_sb_a = wpool.tile([P, wsplit], mybir.dt.bfloat16)
            w_sb_b = wpool.tile([P, ncols - wsplit], mybir.dt.bfloat16)
            nc.sync.dma_start(out=w_sb_a, in_=w_d[:, :wsplit])
            nc.sync.dma_start(out=w_sb_b, in_=w_d[:, wsplit:])

            def wblk(bid, r, c0, c1):
                off = offs[bid]
                if off + c1 <= wsplit:
                    return w_sb_a[r[0]:r[1], off + c0: off + c1]
                return w_sb_b[r[0]:r[1], off - wsplit + c0: off - wsplit + c1]

            def mm(out_ap, lhsT, rhs, start, stop):
                inst = nc.tensor.matmul(out_ap, lhsT, rhs, start=start, stop=stop)
                if _PREV_MM[0] is not None:
                    tile.add_dep_helper(inst.ins, _PREV_MM[0].ins, sync=False,
                                        reason="pe order")
                _PREV_MM[0] = inst
                return inst

            for b in [bb for _ in range(repeat) for bb in range(BPC)]:
                base = b * L
                # ---- input DMA: 8 groups of 4 chunks + chunk 32 ----
                xg = []
                for g in range(8):
                    t_ = xpool.tile([P, 4, C], mybir.dt.bfloat16)
                    if g == 0:
                        nc.sync.dma_start(out=t_[64:128, 0, :], in_=x_d[base: base + 64, :])
                        nc.sync.dma_start(
                            out=t_[:, 1:4, :],
                            in_=x_d[base + 64: base + 448, :].rearrange(
                                "(c p) n -> p c n", p=P))
                    else:
                        nc.sync.dma_start(
                            out=t_,
                            in_=x_d[base + 512 * g - 64: base + 512 * g + 448, :].rearrange(
                                "(c p) n -> p c n", p=P))
                    xg.append(t_)
                t_ = xpool.tile([P, 1, C], mybir.dt.bfloat16)
                nc.sync.dma_start(out=t_[0:64, 0, :], in_=x_d[base + L - 64: base + L, :])
                xg.append(t_)

                def chunk(k, r):
                    return xg[k // 4][r[0]:r[1], k % 4, :]

                # ---- pooling: accumulate chunk contributions into 2 halves ----
                ph = [pspool.tile([P, 512], mybir.dt.float32, name=f"ph{h}")
                      for h in range(2)]
                pooled = [None, None]
                for (k, r, bid, h, st, sp) in pool_sched:
                    mm(ph[h][:, 0:C], wblk(bid, r, 0, P), chunk(k, r),
                       start=st, stop=sp)
                    if sp:
                        pooled[h] = plpool.tile([P, C], mybir.dt.bfloat16, name=f"pooled{h}")
                        nc.vector.tensor_copy(pooled[h], ph[h][:, 0:C])

                # ---- main tiles: fine + apply into PSG-grouped psum ----
                for g0 in range(0, NTILES, PSG):
                    gi = b * (NTILES // PSG) + g0 // PSG
                    psg = psout.tile([P, PSG, 512], mybir.dt.float32, name="psg",
                                     tag="psg")
                    tiles = list(range(g0, g0 + PSG))
                    # order: L-run, R-run, apply-run (for LDWEIGHTS sharing)
                    for fi in range(2):
                        for mi, m in enumerate(tiles):
                            bid, r, k = fine_sched[m][fi]
                            mm(psg[:, mi, 0:C], wblk(bid, r, 0, P), chunk(k, r),
                               start=(fi == 0), stop=False)
                    for mi, m in enumerate(tiles):
                        pieces = apply_sched[m]
                        for pi, (bid, h) in enumerate(pieces):
                            mm(psg[:, mi, 0:C],
                               wblk(bid, (0, P), 0, P), pooled[h],
                               start=False, stop=(pi == len(pieces) - 1))

                    # ---- evacuate + out-DMA ----
                    if g0 % OGRP == 0:
                        og = opool.tile([P, OGRP, C], mybir.dt.bfloat16)
                    osl = og[:, g0 % OGRP: g0 % OGRP + PSG, :]
                    if ACT_EVERY and gi % ACT_EVERY == ACT_EVERY - 1:
                        nc.scalar.copy(osl, psg[:, :, 0:C])
                    else:
                        nc.vector.tensor_copy(osl, psg[:, :, 0:C])
                    if (g0 + PSG) % OGRP == 0:
                        o0 = g0 + PSG - OGRP
                        dst = out_d[base + o0 * P: base + (o0 + OGRP) * P, :]
                        nc.sync.dma_start(out=dst.rearrange("(c p) n -> p c n", p=P),
                                          in_=og)
    _dedupe_ldweights(nc)
    nc.compile()
    return nc


_CACHE = {}


def _get_state(repeat=1):
    key = ("nc", repeat)
    if key not in _CACHE:
        if "blocks" not in _CACHE:
            _CACHE["blocks"] = _build_weights_and_schedule()
        nc = _build_program(*_CACHE["blocks"], repeat=repeat)
        _CACHE[key] = nc
    return _CACHE[key], _CACHE["blocks"][0].flat()


def run(x, **spmd_kwargs):
    """Returns (out [B,L,C] fp32, BassKernelResults)."""
    x = np.asarray(x)
    nc, wf = _get_state()
    in_maps = []
    for core in range(N_CORES):
        xs = np.ascontiguousarray(x[core * BPC: (core + 1) * BPC]).reshape(BPC * L, C)
        in_maps.append({"x": xs.astype(BF16), "w": wf})
    res = run_bass_kernel_spmd(nc, in_maps, list(range(N_CORES)), **spmd_kwargs)
    outs = [np.asarray(res.results[i]["out"]).reshape(BPC, L, C) for i in range(N_CORES)]
    return np.concatenate(outs, axis=0).astype(np.float32), res


def kernel(x):
    return run(x)[0]
